# revision 41
# baseline (speedup 1.0000x reference)
"""Trainium2 Bass kernel for nn_AttentionBlock (B=8, C=128, H=W=64, A=16).

Data-parallel over batch across 8 NeuronCores (one batch each). Per core,
attention over N=4096 pixels, A=16 attention channels:

  xf[C,N] -> q,k [A+1,N] bf16 (17th "bias channel": q row = const g,
  k row = 1, so S' = q^T k = S + g), vT in fp8e5 DoubleRow pair layout.

Pipeline (v2):
  - head: weights packed into one DMA ([128,384] bf16) + biases ([128,3]
    f32); the 8 x chunks issued from 4 different engine DGE queues so all
    transfers are in flight by ~6us; ~10 warmup matmuls on a memset tile
    push the PE HAM clock-gate to 2.4GHz before real work arrives.
  - projection phase computes k, q AND all 32 v tiles (bf16, no perf-mode
    switches), with PSUM pool bufs=6 for cross-iteration overlap. v drains
    to fp8 via GpSimd, k/q bias-adds on DVE/ScalarE.
  - per 512-query chunk, 32 key tiles in 11 steps of 3 (S_ps [128,1536]
    = 3 PSUM banks, double-buffered; O + Z accumulators take the other 2):
    S'^T step: 3 concurrent K=17 bf16 matmuls (rotating tile_position
    row groups). P = exp(S - 4) -> fp8e5 arena (double-buffered across
    chunks), engine alternating per step (6 ScalarE / 5 DVE):
      ScalarE: activation Exp (bias -4-g) -> float8e5
      DVE:     Schraudolph bits uint8(min(5.7708*S', 123)) == fp8e5(exp)
  - PV and Z pairs run LAG steps behind through a queue, drained in
    batches of up to 6 pairs every OTHER step (halves the bf16<->DR
    PE mode transitions), Z matmuls grouped before O matmuls so runs of
    identical/DR weights pipeline back-to-back:
      O(c) += vT2_p^T @ P_p      Z(c) += ones^T @ P_p   (fp8e5 DoubleRow)
  - normalize: out = O * recip(Z) + (x + bv); last chunk split in half so
    the store overlaps the second half's normalize.

fp8 e5m2 for P/V: value range fits (max exp ~2.1e3 << 57344); bits >=
0x7C are inf/NaN to the PE so the DVE path clamps at bits 123.

HAM clock-gate management: the PE defaults to 1.2GHz and needs ~3.4us
of sustained matmul activity to reach 2.4GHz; it re-throttles after
idle/sparse windows. Warmup matmuls (8x N=512 on a memset tile) start
the window before x arrives; graduated junk-matmul fillers (4 at the
proj->main PSUM handoff + 5/5/5/3/3 over chunk 0's first five steps,
written into the not-yet-live O/Z banks) hold the clock at 2.4GHz
through chunk 0's exp-gated ramp, where the PV pipeline is still
empty. With these, the trace shows one continuous warm span over the
whole kernel.

Measured rel err 3.03e-3 (tolerance 2e-2); HW exec 137.7-138.9us
(mean ~138.1 over 5 runs) vs 147.5us for the session-start baseline
on the same device.
"""

import os
import numpy as np

import concourse.bass as bass
import concourse.mybir as mybir
import concourse.tile as tile
from concourse import bacc
from concourse.bass_utils import run_bass_kernel_spmd

try:
    import ml_dtypes

    _BF16 = np.dtype(ml_dtypes.bfloat16)
except ImportError:  # pragma: no cover
    _BF16 = None

N_CORES = 8
C = 128
A = 16
A1 = A + 1          # +1 bias channel
B = 8
HW = 64
IC = 512            # query-chunk width (one PSUM bank)

A5 = 4.0 / np.log(2.0)          # fp8e5 Schraudolph scale (5.7708)
BSH = 4.0                       # exp shift: P = exp(s - BSH)
GCH = (60.0 - BSH * A5) / A5    # bias-channel constant g (6.3973)
CLIP = 123.0                    # max fp8e5 bits (0x7B = 57344)


def build_nc(n=4096):
    f32 = mybir.dt.float32
    bf16 = mybir.dt.bfloat16
    fp8 = mybir.dt.float8e5
    u8 = mybir.dt.uint8
    Ident = mybir.ActivationFunctionType.Identity
    Exp = mybir.ActivationFunctionType.Exp
    DR = mybir.MatmulPerfMode.DoubleRow
    Alu = mybir.AluOpType

    nj = n // 128        # 32 key tiles
    npair = nj // 2      # 16
    ni = n // IC         # 8 query chunks
    nx = n // 512        # x chunks
    nstep = (nj + 2) // 3
    # engine per step: 1 = ScalarE, 0 = DVE (6/5; short step 10 to DVE).
    # NOTE: the double-Scalar at steps 8,9 is deliberate -- its consumers
    # (S matmuls of s10 / next chunk's s1) sit in drain-padded slots, and
    # the chunk BOUNDARY stays engine-alternating (s10 DVE -> s0' Scalar).
    # A "balanced" [..,1,0,0] variant measured 4us slower: its double-DVE
    # at s9,s10 delays the exp gating the next chunk's first S matmuls.
    seng = [1, 0, 1, 0, 1, 0, 1, 0, 1, 1, 0]
    LAG = int(os.environ.get("BASS_LAG", "4"))     # pvz lag in steps
    PVS = int(os.environ.get("BASS_PVS", "6"))     # max pairs per drain
    DRP = int(os.environ.get("BASS_DRP", "2"))     # drain period in steps
    NWARM = int(os.environ.get("BASS_NWARM", "8"))  # warmup matmuls

    nc = bacc.Bacc("TRN2", target_bir_lowering=False, debug=False,
                   num_devices=N_CORES)

    xbf_ext = nc.dram_tensor("x_bf", [C, n], bf16, kind="ExternalInput").ap()
    wpk_ext = nc.dram_tensor("wpack", [C, 3 * C], bf16,
                             kind="ExternalInput").ap()
    bpk_ext = nc.dram_tensor("bpack", [C, 3], f32, kind="ExternalInput").ap()
    out_ext = nc.dram_tensor("out", [C, n], f32, kind="ExternalOutput").ap()

    with tile.TileContext(nc) as tc:
        with tc.tile_pool(name="persist", bufs=1) as persist:
            wpack = persist.tile([C, 3 * C], bf16, tag="wpack")
            bpack = persist.tile([C, 3], f32, tag="bpack")
            wk4 = wpack[:, 0:C]
            wq4 = wpack[:, C:2 * C]
            wvT = wpack[:, 2 * C:3 * C]
            bk4_sb = bpack[:, 0:1]
            bq4_sb = bpack[:, 1:2]
            bv_sb = bpack[:, 2:3]
            xf_bf = persist.tile([C, n], bf16, tag="xf_bf")
            warm = persist.tile([C, 512], bf16, tag="warm")

            # --- early DMA issue, spread across engine DGE queues
            # (only gpsimd / sync / scalar can initiate DMAs) ---
            nc.gpsimd.memset(warm[:], 0.0)
            nc.scalar.dma_start(wpack[:], wpk_ext[:])
            nc.sync.dma_start(bpack[:], bpk_ext[:])
            for h in range(nx):
                sl = slice(h * 512, (h + 1) * 512)
                eng = (nc.gpsimd, nc.scalar, nc.sync, nc.gpsimd,
                       nc.scalar, nc.sync, nc.gpsimd, nc.sync)[h]
                eng.dma_start(xf_bf[:, sl], xbf_ext[:, sl])

            ones2 = persist.tile([C, 256], fp8, tag="ones2")
            nc.vector.memset(ones2[:], 1.0)
            negb = persist.tile([C, 1], f32, tag="negb")
            nc.vector.memset(negb[:], -(BSH + GCH))

            q4 = persist.tile([C, n], bf16, tag="q4")
            k4 = persist.tile([C, n], bf16, tag="k4")
            vT2 = persist.tile([C, n], fp8, tag="vT2")
            arena0 = persist.tile([C, nj * 512], fp8, tag="arena0",
                                  name="arena0")
            arena1 = persist.tile([C, nj * 512], fp8, tag="arena1",
                                  name="arena1")
            arenas = [arena0, arena1]

            # --- warmup: wake the PE HAM clock gate before x arrives ---
            with tc.tile_pool(name="warm_ps", bufs=1, space="PSUM") as wps:
                wp = wps.tile([C, 512], f32, tag="wp")
                for _ in range(NWARM):
                    nc.tensor.matmul(wp[:], warm[:, :C], warm[:],
                                     start=True, stop=True)

            # --- projection phase: k, q and v (all bf16 matmuls) ---
            with tc.tile_pool(name="proj_ps", bufs=6, space="PSUM") as pps:
                for h in range(nx):
                    sl = slice(h * 512, (h + 1) * 512)
                    kp = pps.tile([C, 512], f32, tag="qkp")
                    nc.tensor.matmul(kp[:], wk4, xf_bf[:, sl],
                                     start=True, stop=True)
                    nc.vector.tensor_scalar_add(k4[:, sl], kp[:], bk4_sb)
                    qp = pps.tile([C, 512], f32, tag="qkp")
                    nc.tensor.matmul(qp[:], wq4, xf_bf[:, sl],
                                     start=True, stop=True)
                    nc.scalar.activation(q4[:, sl], qp[:], Ident,
                                         bias=bq4_sb)
                    if h < 4:
                        # v tiles 16-31 ride chunk 0's filler slots
                        # instead (see main loop) - lightens the
                        # drain-paced projection phase
                        vp = pps.tile([C, 512], f32, tag="qkp")
                        for r in range(4):
                            jt = 4 * h + r
                            nc.tensor.matmul(
                                vp[:, 128 * r:128 * r + 128],
                                xf_bf[:, jt * 128:(jt + 1) * 128],
                                wvT, start=True, stop=True)
                        # GpSimd has no PSUM port: split the fp8 drain
                        # across ScalarE and DVE
                        nc.scalar.activation(
                            vT2[:, h * 512:h * 512 + 256],
                            vp[:, 0:256], Ident)
                        nc.vector.tensor_copy(
                            vT2[:, h * 512 + 256:(h + 1) * 512],
                            vp[:, 256:512])

            # --- main attention loop ---
            with tc.tile_pool(name="ep_pool", bufs=2) as epp, \
                 tc.tile_pool(name="ps_S", bufs=2, space="PSUM") as psS, \
                 tc.tile_pool(name="ps_O", bufs=1, space="PSUM") as psO, \
                 tc.tile_pool(name="ps_Z", bufs=1, space="PSUM") as psZ:

                oz = {}          # chunk -> (O_ps, Z_ps)
                queue = []       # (ready_gstep, chunk, pair)
                state = {"g": 0}

                def normalize(c, split):
                    O_ps, Z_ps = oz.pop(c)
                    nh = 2 if split else 1
                    w = IC // nh
                    for hf in range(nh):
                        isl = slice(c * IC + hf * w, c * IC + (hf + 1) * w)
                        psl = slice(hf * w, (hf + 1) * w)
                        recip = epp.tile([C, IC], f32, tag="recip")
                        nc.vector.reciprocal_approx_fast(recip[:, psl],
                                                         Z_ps[:, psl])
                        xr = epp.tile([C, IC], bf16, tag="xr")
                        nc.scalar.activation(xr[:, psl], xf_bf[:, isl],
                                             Ident, bias=bv_sb)
                        o1 = epp.tile([C, IC], f32, tag="o1")
                        nc.vector.tensor_mul(o1[:, psl], O_ps[:, psl],
                                             recip[:, psl])
                        o2 = epp.tile([C, IC], f32, tag="o2")
                        if split and hf == 0:
                            # first half's add on fast DVE, store via
                            # gpsimd queue; second half add on gpsimd
                            # overlaps the first store
                            nc.vector.tensor_add(o2[:, psl], o1[:, psl],
                                                 xr[:, psl])
                            nc.gpsimd.dma_start(out_ext[:, isl],
                                                o2[:, psl])
                        else:
                            nc.gpsimd.tensor_add(o2[:, psl], o1[:, psl],
                                                 xr[:, psl])
                            nc.sync.dma_start(out_ext[:, isl], o2[:, psl])

                def pv_z(c, p, which):
                    O_ps, Z_ps = oz[c]
                    ar = arenas[c % 2]
                    pt3 = ar[:, 1024 * p:1024 * p + 1024] \
                        .rearrange("p (k x) -> p k x", k=2)
                    if which == 0:
                        o3 = ones2[:].rearrange("p (k x) -> p k x", k=2)
                        nc.tensor.matmul(Z_ps[:], o3, pt3, start=p == 0,
                                         stop=p == npair - 1, perf_mode=DR)
                    else:
                        v3 = vT2[:, 256 * p:256 * p + 256] \
                            .rearrange("p (k x) -> p k x", k=2)
                        nc.tensor.matmul(O_ps[:], v3, pt3, start=p == 0,
                                         stop=p == npair - 1, perf_mode=DR)

                def drain(limit):
                    batch = []
                    while queue and queue[0][0] <= state["g"] \
                            and len(batch) < limit:
                        _, c, p = queue[0]
                        if p == 0 and c > 0 and batch:
                            # never mix chunks in one batch: the new
                            # chunk's O/Z bank reuse (bufs=1) must be
                            # emitted after the previous chunk's
                            # normalize reads, or the schedule can race
                            break
                        queue.pop(0)
                        if p == 0 and c > 0:
                            oz[c] = (psO.tile([C, IC], f32, tag="O_ps",
                                              name=f"O_ps{c}"),
                                     psZ.tile([C, IC], f32, tag="Z_ps",
                                              name=f"Z_ps{c}"))
                        batch.append((c, p))
                    for c, p in batch:
                        pv_z(c, p, 0)
                    for c, p in batch:
                        pv_z(c, p, 1)
                    for c, p in batch:
                        if p == npair - 1:
                            normalize(c, split=(c == ni - 1))

                for ic in range(ni):
                    isl = slice(ic * IC, (ic + 1) * IC)
                    arena = arenas[ic % 2]
                    pdone = 0
                    if ic == 0:
                        oz[0] = (psO.tile([C, IC], f32, tag="O_ps",
                                          name="O_ps0"),
                                 psZ.tile([C, IC], f32, tag="Z_ps",
                                          name="Z_ps0"))
                        # filler matmuls bridge the proj->main PSUM
                        # handoff gap so the HAM clock gate stays warm
                        for dmy in range(4):
                            nc.tensor.matmul(oz[0][dmy % 2][:],
                                             warm[:, :C], warm[:],
                                             start=True, stop=True)
                    for s in range(nstep):
                        j0 = 3 * s
                        tl = min(3, nj - j0)
                        S_ps = psS.tile([128, 1536], f32, tag="S_ps")
                        for r in range(tl):
                            jt = j0 + r
                            p0 = 32 * (jt % 4)
                            nc.tensor.matmul(
                                S_ps[:, r * 512:r * 512 + 512],
                                k4[p0:p0 + A1, jt * 128:(jt + 1) * 128],
                                q4[p0:p0 + A1, isl],
                                start=True, stop=True,
                                tile_position=(p0, 0))
                        if ic == 0 and s < 5:
                            # chunk 0's PV pipeline is empty for the
                            # first LAG+1 steps (exp-gated, PE mostly
                            # idle): keep PE duty high with junk bf16
                            # matmuls into the not-yet-live O/Z banks
                            # (pair 0's start=True overwrites at s=5),
                            # and compute v tiles 16-31 in the same
                            # slots (junk-then-real WAW on bank s%2;
                            # the bank's v-copy lands 2 steps before
                            # its next reuse)
                            bank = oz[0][s % 2]
                            for dmy in range(3 if s < 4 else 4):
                                nc.tensor.matmul(bank[:],
                                                 warm[:, :C], warm[:],
                                                 start=True, stop=True)
                            if s < 4:
                                for r in range(4):
                                    jt = 16 + 4 * s + r
                                    nc.tensor.matmul(
                                        bank[:, 128 * r:128 * r + 128],
                                        xf_bf[:, jt * 128:(jt + 1) * 128],
                                        wvT, start=True, stop=True)
                                vsl = slice(2048 + 512 * s,
                                            2048 + 512 * (s + 1))
                                if seng[s]:
                                    nc.vector.tensor_copy(vT2[:, vsl],
                                                          bank[:])
                                else:
                                    nc.scalar.activation(vT2[:, vsl],
                                                         bank[:], Ident)
                        if s % DRP == DRP - 1 or s == nstep - 1:
                            drain(PVS)
                        dst = arena[:, j0 * 512:(j0 + tl) * 512]
                        if seng[s]:
                            nc.scalar.activation(dst, S_ps[:, :tl * 512],
                                                 Exp, bias=negb[:])
                        else:
                            nc.vector.tensor_scalar(
                                dst.bitcast(u8), S_ps[:, :tl * 512],
                                A5, CLIP, Alu.mult, Alu.min)
                        state["g"] += 1
                        lag = LAG if ic < ni - 1 else 1
                        while (pdone + 1) * 2 <= j0 + tl:
                            queue.append((state["g"] + lag, ic, pdone))
                            pdone += 1
                state["g"] = 1 << 30
                while queue:
                    drain(PVS)

    nc.compile()
    return nc


_NC_CACHE = {}


def _get_nc(n=4096):
    if n not in _NC_CACHE:
        _NC_CACHE[n] = build_nc(n)
    return _NC_CACHE[n]


def _spread(w):
    """[A, C] weight -> [C, C] lhsT with W.T in 4 row-group column bands
    (17th column of each band = 0: the bias channel comes from the bias)."""
    out = np.zeros((C, C), dtype=np.float32)
    for r in range(4):
        out[:, 32 * r:32 * r + A] = w.T
    return out


def _spread_bias(b, ch):
    out = np.zeros((C,), dtype=np.float32)
    for r in range(4):
        out[32 * r:32 * r + A] = b
        out[32 * r + A] = ch
    return out


def kernel(x, Wq, bq, Wk, bk, Wv, bv):
    x = np.asarray(x, dtype=np.float32)
    Wq = np.asarray(Wq, dtype=np.float32)
    bq = np.asarray(bq, dtype=np.float32)
    Wk = np.asarray(Wk, dtype=np.float32)
    bk = np.asarray(bk, dtype=np.float32)
    Wv = np.asarray(Wv, dtype=np.float32)
    bv = np.asarray(bv, dtype=np.float32)

    b, c, hh, ww = x.shape
    n = hh * ww
    assert (b, c) == (B, C) and n == 4096

    nc = _get_nc(n)

    wpack = np.concatenate(
        [_spread(Wk), _spread(Wq), np.ascontiguousarray(Wv.T)],
        axis=1).astype(_BF16)
    bpack = np.stack(
        [_spread_bias(bk, 1.0), _spread_bias(bq, GCH),
         bv.astype(np.float32)], axis=1).astype(np.float32)
    in_common = {
        "wpack": np.ascontiguousarray(wpack),
        "bpack": np.ascontiguousarray(bpack),
    }
    in_maps = []
    for i in range(B):
        xi = np.ascontiguousarray(x[i].reshape(C, n))
        in_maps.append({"x_bf": xi.astype(_BF16), **in_common})

    trace = bool(int(os.environ.get("BASS_KERNEL_PROFILE", "0")))
    res = run_bass_kernel_spmd(nc, in_maps, core_ids=list(range(N_CORES)),
                               trace=trace)
    if trace:
        kernel.last_exec_time_ns = res.exec_time_ns
        kernel.last_results = res
    out = np.stack([res.results[i]["out"].reshape(C, hh, ww)
                    for i in range(B)])
    return out
